# revision 1
# baseline (speedup 1.0000x reference)
"""Trainium2 Bass kernel for BoxMultiHeadedAttention (B=8, N=512, D=512, H=8).

Sharding: data-parallel over batch — each of the 8 NeuronCores computes one
batch element end-to-end; weights replicated; no collectives.

Per-core algorithm (transposed-attention layout [m(part), n(free)]):
  * q/k/v projections on PE (bf16) from DMA-transposed inputs.
  * scoresT = kT_h.T @ qT_h (1/8 folded into k); E = exp(scoresT + maskcol)
    on ACT (constant stability shift baked into maskcol).
  * geometry wg:
      - dx/dy: symmetric ln field on ACT; phase fractions
        t = (alpha_j/4pi) * dx2 replicated onto partitions by one-hot-scaled
        selector matmuls (exact f32), folded to [-1/2,1/2) by DVE
        magic-number round, then Sin on ACT (cos = sin(pi/2 - 2pi|f|));
        WG contraction on PE (bf16) with h-major output columns.
      - dw/dh: exactly separable (angle addition) -> rank-64 PE contraction
        of per-box sin/cos banks (phases folded the same way).
  * exp-domain softmax: T = E*(1 + obj_n*wgd), wgd = (max(wg+bG,1e-6)-1)*obj_m;
    row sums via PE ones-matmul; 1/s applied at AV eviction; final linear on
    PE from the transposed AV result.
"""
import math
import numpy as np
from contextlib import ExitStack

import concourse.bass as bass
import concourse.mybir as mybir
import concourse.tile as tile
from concourse.bass_utils import run_bass_kernel_spmd

F32 = mybir.dt.float32
BF16 = mybir.dt.bfloat16
AF = mybir.ActivationFunctionType
ALU = mybir.AluOpType

B, N, D, H = 8, 512, 512, 8
DK = D // H
P = 128
NRB = N // P
NG = 8
GM = 16
WAVE_LEN = 1000.0
MAGIC = 12582912.0
C2 = float(2.0 * math.log(0.001))
ESHIFT = -6.0
TWO_PI = float(2.0 * math.pi)
HALF_PI = float(math.pi / 2.0)
PI_ = float(math.pi)

_alphas = (100.0 / (WAVE_LEN ** (np.arange(8) / 8.0))).astype(np.float64)


def _split_multi_waits(nc):
    """walrus here accepts only ONE sync-wait per ISA instruction; hoist
    extras onto NoOps inserted before the offending instruction."""
    n_fix = 0
    for blk in nc.main_func.blocks:
        insts = list(blk.instructions)
        out, dirty = [], False
        for inst in insts:
            si = inst.sync_info
            waits = list(si.on_wait) if si is not None else []
            if len(waits) > 1:
                for kk, w in enumerate(waits[:-1]):
                    out.append(mybir.InstNoOp(
                        name=f"I-waitfix-{n_fix}-{kk}", engine=inst.engine,
                        sync_info=mybir.SyncInfo(on_wait=[w], on_update=[])))
                inst.sync_info = mybir.SyncInfo(
                    on_wait=[waits[-1]], on_update=list(si.on_update))
                n_fix += 1
                dirty = True
            out.append(inst)
        if dirty:
            blk.instructions = out
    return n_fix


def _selector_const():
    # SELAP[64*W + q*16 + m_loc, q, m_loc*8 + j] = alpha_j/(4pi)
    selap = np.zeros((P, 4, P), dtype=np.float32)
    for W in range(2):
        for q in range(4):
            for m_loc in range(GM):
                for j in range(8):
                    selap[64 * W + q * 16 + m_loc, q, m_loc * 8 + j] = \
                        _alphas[j] / (4.0 * math.pi)
    return selap


def _onehot8():
    # OH8[p, h, c] = 1.0 iff c == h  (lhsT column-one-hot for row sums)
    oh = np.zeros((P, H, H), dtype=np.float32)
    for h in range(H):
        oh[:, h, h] = 1.0
    return oh


def _wg_consts(WG, bG):
    out = {}
    # double-angle features: fsin_tile = sin(pi f)cos(pi f)  (weight 2*WGs),
    # fcos_tile = sin^2(pi f)                  (weight -2*WGc, const +WGc)
    gmap = [lambda j: j, lambda j: 32 + j, lambda j: 8 + j, lambda j: 40 + j]
    gscl = [2.0, -2.0, 2.0, -2.0]
    wblk = np.zeros((4, P, P), dtype=np.float32)
    for c in range(4):
        for m_loc in range(GM):
            for j in range(8):
                for h in range(H):
                    wblk[c, m_loc * 8 + j, h * GM + m_loc] = \
                        gscl[c] * WG[h, gmap[c](j)]
    out["WBLK"] = wblk

    acol = np.zeros((64, 1), np.float32)
    pcol_m = np.zeros((64, 1), np.float32)
    pcol_n = np.zeros((64, 1), np.float32)
    w1 = np.zeros((64, H), np.float32)
    for f in range(2):
        for j in range(8):
            gs = 16 + 8 * f + j
            gc = 48 + 8 * f + j
            a = _alphas[j] / (4.0 * math.pi)
            for t in range(4):
                k = (f * 8 + j) * 4 + t
                acol[k, 0] = a
                pcol_m[k, 0] = 0.25 if t in (0, 2) else 0.0
                if t == 0:
                    pcol_n[k, 0] = 0.0; w1[k] = WG[:, gs]
                elif t == 1:
                    pcol_n[k, 0] = 0.75; w1[k] = WG[:, gs]   # -cos -> +pi
                elif t == 2:
                    pcol_n[k, 0] = 0.25; w1[k] = WG[:, gc]
                else:
                    pcol_n[k, 0] = 0.0; w1[k] = WG[:, gc]
    out["ACOL"] = acol
    out["PCOL_M"], out["PCOL_N"] = pcol_m, pcol_n
    out["W1E"] = np.repeat(w1, GM, axis=1).astype(np.float32)
    # bG' = bG + sum_j (WGc_x + WGc_y)  (the "+1" of cos = 1 - 2 sin^2)
    bg2 = bG.astype(np.float64) + WG[:, 32:48].sum(axis=1)
    out["BGCOL"] = np.repeat(bg2.astype(np.float32), GM)[:, None]
    return out


def _host_prep(inputs):
    q = np.asarray(inputs["input_query"], np.float32)
    k = np.asarray(inputs["input_key"], np.float32)
    v = np.asarray(inputs["input_value"], np.float32)
    box = np.asarray(inputs["input_box"], np.float32)
    mask = np.asarray(inputs["mask"])
    nobj = np.asarray(inputs["not_objects"])
    WG = np.asarray(inputs["WG"], np.float32)
    bG = np.asarray(inputs["bG"], np.float32)
    wgc = _wg_consts(WG, bG)
    sela = _selector_const()

    x_min, y_min, x_max, y_max = [box[..., i] for i in range(4)]
    cx = (x_min + x_max) * 0.5
    cy = (y_min + y_max) * 0.5
    ww = x_max - x_min + 1.0
    hh = y_max - y_min + 1.0
    l2w = (2.0 * np.log(ww)).astype(np.float32)
    l2h = (2.0 * np.log(hh)).astype(np.float32)

    maskcol = (np.where(mask == 0, -1e9, 0.0) + ESHIFT).astype(np.float32)
    obj = (1.0 - nobj.astype(np.float32)).astype(np.float32)

    shared = {
        "Wq": np.asarray(inputs["Wq"], np.float32),
        "Wk": np.asarray(inputs["Wk"], np.float32),
        "Wv": np.asarray(inputs["Wv"], np.float32),
        "Wo": np.asarray(inputs["Wo"], np.float32),
        "bqcol": np.asarray(inputs["bq"], np.float32).reshape(NRB, P).T.copy(),
        "bk8col": (np.asarray(inputs["bk"], np.float32) * 8.0
                   ).reshape(NRB, P).T.copy(),
        "bvrow": np.asarray(inputs["bv"], np.float32),
        "borow": np.asarray(inputs["bo"], np.float32),
        "SELAP": sela, "IDENT": np.eye(P, dtype=np.float32),
        "ONEHOT8": _onehot8(),
        "WBLK": wgc["WBLK"], "W1E": wgc["W1E"],
        "BGCOL": wgc["BGCOL"], "ACOL": wgc["ACOL"],
        "PCOL_M": wgc["PCOL_M"], "PCOL_N": wgc["PCOL_N"],
    }
    in_maps = []
    for b in range(B):
        m = dict(shared)
        m.update({
            "xq": q[b].copy(), "xk": k[b].copy(), "xv": v[b].copy(),
            "cxrow": cx[b].copy(), "cyrow": cy[b].copy(),
            "cxcol": cx[b].reshape(NRB, P).T.copy(),
            "cycol": cy[b].reshape(NRB, P).T.copy(),
            "l2wrow": l2w[b].copy(), "l2hrow": l2h[b].copy(),
            "mcol": maskcol[b].reshape(NRB, P).T.copy(),
            "objrow": obj[b].copy(),
            "ocol": obj[b].reshape(NRB, P).T.copy(),
        })
        in_maps.append(m)
    return in_maps


def build_nc():
    nc = bass.Bass()

    def dp(name, shape):
        return nc.declare_dram_parameter(name, list(shape), F32, isOutput=False)

    xq = dp("xq", (N, D)); xk = dp("xk", (N, D)); xv = dp("xv", (N, D))
    Wq = dp("Wq", (D, D)); Wk = dp("Wk", (D, D)); Wv = dp("Wv", (D, D))
    Wo = dp("Wo", (D, D))
    bqcol = dp("bqcol", (P, NRB)); bk8col = dp("bk8col", (P, NRB))
    bvrow = dp("bvrow", (D,)); borow = dp("borow", (D,))
    cxrow = dp("cxrow", (N,)); cyrow = dp("cyrow", (N,))
    cxcol = dp("cxcol", (P, NRB)); cycol = dp("cycol", (P, NRB))
    l2wrow = dp("l2wrow", (N,)); l2hrow = dp("l2hrow", (N,))
    mcol = dp("mcol", (P, NRB)); objrow = dp("objrow", (N,))
    ocol = dp("ocol", (P, NRB))
    SELAP = dp("SELAP", (P, 4, P)); IDENT = dp("IDENT", (P, P))
    ONEHOT8 = dp("ONEHOT8", (P, H, H))
    WBLK = dp("WBLK", (4, P, P)); W1E = dp("W1E", (64, P))
    BGCOL = dp("BGCOL", (P, 1))
    ACOL = dp("ACOL", (64, 1))
    PCOL_M = dp("PCOL_M", (64, 1)); PCOL_N = dp("PCOL_N", (64, 1))
    out = nc.declare_dram_parameter("out", [N, D], F32, isOutput=True)
    rs_dram = nc.dram_tensor("rs_scratch", [H, N], F32)

    with ExitStack() as ctx:
        tc = ctx.enter_context(tile.TileContext(nc))
        const = ctx.enter_context(tc.tile_pool(name="const", bufs=1))
        persist = ctx.enter_context(tc.tile_pool(name="persist", bufs=1))

        # ---------------- constants ----------------
        with tc.tile_pool(name="cwork", bufs=2) as cwork:
            selap_f = cwork.tile([P, 4, P], F32, tag="selapf")
            nc.sync.dma_start(selap_f[:], SELAP[:])
            selap_t = const.tile([P, 4, P], F32, tag="selap")
            nc.vector.tensor_copy(selap_t[:], selap_f[:])
            oh8_f = cwork.tile([P, H, H], F32, tag="oh8f")
            nc.sync.dma_start(oh8_f[:], ONEHOT8[:])
            oh8_t = const.tile([P, H, H], BF16, tag="oh8")
            nc.vector.tensor_copy(oh8_t[:], oh8_f[:])
            ident_t = const.tile([P, P], F32, tag="ident")
            nc.sync.dma_start(ident_t[:], IDENT[:])
            wblk_t4 = []
            for c in range(4):
                wf = cwork.tile([P, P], F32, tag="wblkf")
                nc.sync.dma_start(wf[:], WBLK[c])
                wb = const.tile([P, P], BF16, tag=f"wblkb{c}")
                nc.vector.tensor_copy(wb[:], wf[:])
                wblk_t4.append(wb)
            w1e_f = const.tile([64, P], F32, tag="w1e")
            nc.sync.dma_start(w1e_f[:], W1E[:])
            bgcol_t = const.tile([P, 1], F32, tag="bgcol")
            nc.sync.dma_start(bgcol_t[:], BGCOL[:])
            bgm1_t = const.tile([P, 1], F32, tag="bgm1")
            nc.vector.tensor_scalar(bgm1_t[:], bgcol_t[:], -1.0, None, ALU.add)
            acol_t = const.tile([64, 1], F32, tag="acol")
            nc.sync.dma_start(acol_t[:], ACOL[:])
            pcolm_t = const.tile([64, 1], F32, tag="pcolm")
            nc.sync.dma_start(pcolm_t[:], PCOL_M[:])
            pcoln_t = const.tile([64, 1], F32, tag="pcoln")
            nc.sync.dma_start(pcoln_t[:], PCOL_N[:])
            ones_bf = const.tile([P, 1], BF16, tag="onesb")
            nc.vector.memset(ones_bf[:], 1.0)
            halfpi_t = const.tile([P, 1], F32, tag="halfpi")
            nc.vector.memset(halfpi_t[:], HALF_PI)
            mcol_t = const.tile([P, NRB], F32, tag="mcol")
            nc.sync.dma_start(mcol_t[:], mcol[:])
            bq_t = const.tile([P, NRB], F32, tag="bq")
            nc.sync.dma_start(bq_t[:], bqcol[:])
            bk8_t = const.tile([P, NRB], F32, tag="bk8")
            nc.sync.dma_start(bk8_t[:], bk8col[:])
            cxcol_t = const.tile([P, NRB], F32, tag="cxcol")
            nc.sync.dma_start(cxcol_t[:], cxcol[:])
            cycol_t = const.tile([P, NRB], F32, tag="cycol")
            nc.sync.dma_start(cycol_t[:], cycol[:])
            ocol_t = const.tile([P, NRB], F32, tag="ocol")
            nc.sync.dma_start(ocol_t[:], ocol[:])
            cxbc = const.tile([P, N], F32, tag="cxbc")
            nc.sync.dma_start(cxbc[:], cxrow[None, :].to_broadcast((P, N)))
            cybc = const.tile([P, N], F32, tag="cybc")
            nc.sync.dma_start(cybc[:], cyrow[None, :].to_broadcast((P, N)))
            l2wbc = const.tile([P, N], F32, tag="l2wbc")
            nc.sync.dma_start(l2wbc[:], l2wrow[None, :].to_broadcast((P, N)))
            l2hbc = const.tile([P, N], F32, tag="l2hbc")
            nc.sync.dma_start(l2hbc[:], l2hrow[None, :].to_broadcast((P, N)))
            objbc_f = cwork.tile([P, N], F32, tag="objbcf")
            nc.sync.dma_start(objbc_f[:], objrow[None, :].to_broadcast((P, N)))
            objbc = const.tile([P, N], BF16, tag="objbc")
            nc.vector.tensor_copy(objbc[:], objbc_f[:])
            bvbc = const.tile([P, D], F32, tag="bvbc")
            nc.sync.dma_start(bvbc[:], bvrow[None, :].to_broadcast((P, D)))
            bobc = const.tile([P, D], F32, tag="bobc")
            nc.sync.dma_start(bobc[:], borow[None, :].to_broadcast((P, D)))

        # ---------------- phase 1: transpose-load + projections ----------------
        xqTb = persist.tile([P, NRB, N], BF16, tag="xqTb")
        xkTb = persist.tile([P, NRB, N], BF16, tag="xkTb")
        xvTb = persist.tile([P, NRB, N], BF16, tag="xvTb")
        wq_b = persist.tile([P, NRB, D], BF16, tag="wqb")
        wk_b = persist.tile([P, NRB, D], BF16, tag="wkb")
        wv_b = persist.tile([P, NRB, D], BF16, tag="wvb")
        wo_b = persist.tile([P, NRB, D], BF16, tag="wob")
        qT = persist.tile([P, NRB, N], BF16, tag="qT")
        kTt = persist.tile([P, NRB, N], BF16, tag="kT")
        v_sb = persist.tile([P, NRB, D], BF16, tag="v_sb")

        with tc.tile_pool(name="work1", bufs=2) as work1, \
             tc.tile_pool(name="psum1", bufs=3, space="PSUM") as psum1:
            for (src, dstb) in ((xq, xqTb), (xk, xkTb), (xv, xvTb)):
                for rb in range(NRB):
                    xrb = work1.tile([P, D], F32, tag="xrb")
                    nc.sync.dma_start(xrb[:], src[rb * P:(rb + 1) * P, :])
                    for cb in range(NRB):
                        tp = psum1.tile([P, P], F32, tag="tp")
                        nc.tensor.transpose(tp[:], xrb[:, cb * P:(cb + 1) * P],
                                            ident_t[:])
                        nc.vector.tensor_copy(
                            dstb[:, cb, rb * P:(rb + 1) * P], tp[:])
            for (Wd, wb_) in ((Wq, wq_b), (Wk, wk_b), (Wv, wv_b), (Wo, wo_b)):
                wf = work1.tile([P, NRB, D], F32, tag="wldf")
                nc.sync.dma_start(wf[:],
                                  Wd.rearrange("(kb p) d -> p kb d", p=P))
                nc.vector.tensor_copy(wb_[:], wf[:])

            for (wb_, xb, dstT) in ((wq_b, xqTb, qT), (wk_b, xkTb, kTt)):
                for ob in range(NRB):
                    ps = psum1.tile([P, N], F32, tag="projps")
                    for kb in range(NRB):
                        nc.tensor.matmul(ps[:],
                                         wb_[:, kb, ob * P:(ob + 1) * P],
                                         xb[:, kb, :],
                                         start=(kb == 0),
                                         stop=(kb == NRB - 1))
                    if dstT is qT:
                        nc.vector.tensor_scalar(dstT[:, ob, :], ps[:],
                                                bq_t[:, ob:ob + 1], None,
                                                ALU.add)
                    else:
                        # kT = (ps + 8*bk) * 0.125
                        nc.vector.tensor_scalar(dstT[:, ob, :], ps[:],
                                                bk8_t[:, ob:ob + 1], 0.125,
                                                ALU.add, ALU.mult)
            for mb in range(NRB):
                ps = psum1.tile([P, D], F32, tag="projps")
                for kb in range(NRB):
                    nc.tensor.matmul(ps[:], xvTb[:, kb, mb * P:(mb + 1) * P],
                                     wv_b[:, kb, :],
                                     start=(kb == 0), stop=(kb == NRB - 1))
                vtmp = work1.tile([P, D], F32, tag="vev")
                nc.vector.tensor_tensor(vtmp[:], ps[:], bvbc[:], ALU.add)
                nc.vector.tensor_copy(v_sb[:, mb, :], vtmp[:])

        # ---------------- phase 2: ln fields ----------------
        dxy2 = persist.tile([P, NRB, 2, N], F32, tag="dxy2")
        with tc.tile_pool(name="work2", bufs=3) as work2:
            for rb in range(NRB):
                for (ci, cbc, ccol, l2bc) in ((0, cxbc, cxcol_t, l2wbc),
                                              (1, cybc, cycol_t, l2hbc)):
                    d_ = work2.tile([P, N], F32, tag="geo_d")
                    nc.vector.tensor_scalar(d_[:], cbc[:], ccol[:, rb:rb + 1],
                                            None, ALU.subtract)
                    d2 = work2.tile([P, N], F32, tag="geo_d2")
                    nc.vector.tensor_tensor(d2[:], d_[:], d_[:], ALU.mult)
                    l2t = work2.tile([P, N], F32, tag="geo_l2")
                    nc.scalar.activation(l2t[:], d2[:], AF.Ln)
                    g_ = work2.tile([P, N], F32, tag="geo_g")
                    nc.vector.tensor_tensor(g_[:], l2t[:], l2bc[:],
                                            ALU.subtract)
                    nc.vector.tensor_scalar_max(dxy2[:, rb, ci, :], g_[:], C2)

        # ---------------- phase 3: dw/dh banks ----------------
        bankM = persist.tile([64, N], BF16, tag="bankM")
        bankN = persist.tile([64, N], BF16, tag="bankN")
        with tc.tile_pool(name="work3", bufs=2) as work3:
            for (pcol, bank) in ((pcolm_t, bankM), (pcoln_t, bankN)):
                t_ = work3.tile([64, N], F32, tag="bk_t")
                nc.vector.tensor_scalar(t_[:32, :], l2wbc[:32, :],
                                        acol_t[:32, :], pcol[:32, :],
                                        ALU.mult, ALU.add)
                nc.vector.tensor_scalar(t_[32:, :], l2hbc[32:64, :],
                                        acol_t[32:, :], pcol[32:, :],
                                        ALU.mult, ALU.add)
                r_ = work3.tile([64, N], F32, tag="bk_r")
                nc.vector.tensor_scalar(r_[:], t_[:], MAGIC, -MAGIC,
                                        ALU.add, ALU.add)
                f_ = work3.tile([64, N], F32, tag="bk_f")
                nc.vector.tensor_tensor(f_[:], t_[:], r_[:], ALU.subtract)
                nc.scalar.activation(bank[:], f_[:], AF.Sin, scale=TWO_PI)

        # ---------------- phase 4: wg ----------------
        wgdT = persist.tile([P, H, NRB, N], BF16, tag="wgdT")
        with tc.tile_pool(name="work4", bufs=3) as work4, \
             tc.tile_pool(name="psum_u", bufs=2, space="PSUM") as psum_u, \
             tc.tile_pool(name="psum_wg", bufs=3, space="PSUM") as psum_wg:
            for rb in range(NRB):
                wgd_il = work4.tile([P, NG, N], BF16, tag="wgd_il")
                for g in range(NG):
                    lhs_wh = work4.tile([64, P], BF16, tag="lhs_wh")
                    mbase = rb * P + g * GM
                    nc.vector.tensor_tensor(
                        lhs_wh[:].rearrange("k (h m) -> k h m", h=H),
                        w1e_f[:].rearrange("k (h m) -> k h m", h=H),
                        bankM[:, mbase:mbase + GM][:, None, :]
                            .to_broadcast((64, H, GM)),
                        ALU.mult)
                    ups = psum_u.tile([P, 2, N], F32, tag="ups")
                    off = 64 * (g // 4)
                    q = g % 4
                    for ci in range(2):
                        nc.tensor.matmul(ups[:, ci, :],
                                         selap_t[off:off + 64, q, :],
                                         dxy2[off:off + 64, rb, ci, :],
                                         start=True, stop=True)
                    rr = work4.tile([P, 2, N], F32, tag="fold_r")
                    nc.vector.tensor_scalar(rr[:], ups[:], MAGIC, -MAGIC,
                                            ALU.add, ALU.add)
                    ff = work4.tile([P, 2, N], F32, tag="fold_f")
                    nc.vector.tensor_tensor(ff[:], ups[:], rr[:],
                                            ALU.subtract)
                    # half-angle: s2 = sin(pi f), c2 = cos(pi f) = sin(pi/2-pi f)
                    s2 = work4.tile([P, 2, N], BF16, tag="s2")
                    nc.scalar.activation(s2[:], ff[:], AF.Sin, scale=PI_)
                    c2 = work4.tile([P, 2, N], BF16, tag="c2")
                    nc.scalar.activation(c2[:], ff[:], AF.Sin, scale=-PI_,
                                         bias=halfpi_t[:])
                    fsin = work4.tile([P, 2, N], BF16, tag="fsin")
                    nc.vector.tensor_tensor(fsin[:], s2[:], c2[:], ALU.mult)
                    fcos = work4.tile([P, 2, N], BF16, tag="fcos")
                    nc.vector.tensor_tensor(fcos[:], s2[:], s2[:], ALU.mult)
                    wgp = psum_wg.tile([P, N], F32, tag="wgp")
                    nc.tensor.matmul(wgp[:], wblk_t4[0][:], fsin[:, 0, :],
                                     start=True, stop=False)
                    nc.tensor.matmul(wgp[:], wblk_t4[1][:], fcos[:, 0, :],
                                     start=False, stop=False)
                    nc.tensor.matmul(wgp[:], wblk_t4[2][:], fsin[:, 1, :],
                                     start=False, stop=False)
                    nc.tensor.matmul(wgp[:], wblk_t4[3][:], fcos[:, 1, :],
                                     start=False, stop=False)
                    nc.tensor.matmul(wgp[:], lhs_wh[:], bankN[:],
                                     start=False, stop=True)
                    # wgd = max(wg + bG', 1e-6) - 1 = max(wg + bG'-1, 1e-6-1)
                    nc.vector.tensor_scalar(wgd_il[:, g, :], wgp[:],
                                            bgm1_t[:], 1e-6 - 1.0,
                                            ALU.add, ALU.max)
                for h in range(H):
                    for g in range(NG):
                        nc.sync.dma_start(
                            wgdT[g * GM:(g + 1) * GM, h, rb, :],
                            wgd_il[h * GM:(h + 1) * GM, g, :])

        # ---------------- phase 5: attention ----------------
        ot = persist.tile([P, NRB, N], BF16, tag="ot")
        with tc.tile_pool(name="work5", bufs=3) as work5, \
             tc.tile_pool(name="psum5", bufs=2, space="PSUM") as psum5, \
             tc.tile_pool(name="psum_s", bufs=2, space="PSUM") as psum_s, \
             tc.tile_pool(name="psum_av", bufs=2, space="PSUM") as psum_av:

            objpair = persist.tile([P, NRB, N], BF16, tag="objpair")
            for rb in range(NRB):
                nc.vector.tensor_scalar(objpair[:, rb, :], objbc[:],
                                        ocol_t[:, rb:rb + 1], None, ALU.mult)
            # head PAIRS (2k, 2k+1) share kT/qT block ob=k at offsets 0/64:
            # one exp + one combine chain over [P, 2, N], shared av bank.
            for ob in range(H // 2):
                h0 = 2 * ob
                av = psum_av.tile([P, N], F32, tag="avps")
                sbank = psum_s.tile([H, N], F32, tag="sbank")
                for rb in range(NRB):
                    st2 = psum5.tile([P, 2, N], F32, tag="stps")
                    for hi in range(2):
                        po = hi * DK
                        nc.tensor.matmul(
                            st2[:, hi, :],
                            kTt[po:po + DK, ob, rb * P:(rb + 1) * P],
                            qT[po:po + DK, ob, :], start=True, stop=True)
                    e_ = work5.tile([P, 2, N], BF16, tag="e_t")
                    nc.scalar.activation(e_[:], st2[:], AF.Exp,
                                         bias=mcol_t[:, rb:rb + 1])
                    e1 = work5.tile([P, 2, N], BF16, tag="e1_t")
                    nc.vector.tensor_tensor(
                        e1[:], e_[:],
                        objpair[:, rb, None, :].to_broadcast((P, 2, N)),
                        ALU.mult)
                    e2 = work5.tile([P, 2, N], BF16, tag="e2_t")
                    nc.vector.tensor_tensor(e2[:], e1[:],
                                            wgdT[:, h0:h0 + 2, rb, :],
                                            ALU.mult)
                    tt_ = work5.tile([P, 2, N], BF16, tag="tt_t")
                    nc.vector.tensor_tensor(tt_[:], e_[:], e2[:], ALU.add)
                    for hi in range(2):
                        po = hi * DK
                        nc.tensor.matmul(sbank[:], oh8_t[:, h0 + hi, :],
                                         tt_[:, hi, :],
                                         start=(rb == 0 and hi == 0),
                                         stop=(rb == NRB - 1 and hi == 1),
                                         skip_group_check=True)
                        nc.tensor.matmul(av[po:po + DK, :],
                                         v_sb[:, rb,
                                              (h0 + hi) * DK:(h0 + hi + 1) * DK],
                                         tt_[:, hi, :], start=(rb == 0),
                                         stop=(rb == NRB - 1),
                                         skip_group_check=True)
                rs = work5.tile([H, N], F32, tag="rs")
                nc.vector.reciprocal(rs[:], sbank[:])
                nc.sync.dma_start(rs_dram[h0:h0 + 2, :], rs[h0:h0 + 2, :])
                rr_b = work5.tile([P, N], F32, tag="rr_b")
                for hi in range(2):
                    nc.sync.dma_start(
                        rr_b[hi * DK:(hi + 1) * DK, :],
                        rs_dram[h0 + hi:h0 + hi + 1, :].to_broadcast((DK, N)))
                nc.vector.tensor_tensor(ot[:, ob, :], av[:], rr_b[:], ALU.mult)

        # final projection: out[n, d]  (own PSUM scope)
        with tc.tile_pool(name="work6", bufs=2) as work6, \
             tc.tile_pool(name="psum6", bufs=2, space="PSUM") as psum6:
            for r in range(NRB):
                ps = psum6.tile([P, D], F32, tag="fps")
                for kt in range(NRB):
                    nc.tensor.matmul(ps[:], ot[:, kt, r * P:(r + 1) * P],
                                     wo_b[:, kt, :],
                                     start=(kt == 0), stop=(kt == NRB - 1))
                fo = work6.tile([P, D], F32, tag="fo")
                nc.vector.tensor_tensor(fo[:], ps[:], bobc[:], ALU.add)
                nc.sync.dma_start(out[r * P:(r + 1) * P, :], fo[:])

    _split_multi_waits(nc)
    return nc


_NC_CACHE = {}


def kernel(**inputs):
    in_maps = _host_prep(inputs)
    if "nc" not in _NC_CACHE:
        _NC_CACHE["nc"] = build_nc()
    nc = _NC_CACHE["nc"]
    res = run_bass_kernel_spmd(nc, in_maps, list(range(B)))
    out = np.stack([res.results[b]["out"] for b in range(B)], axis=0)
    return out.astype(np.float32)


if __name__ == "__main__":
    print("kernel module ok")



# revision 4
# speedup vs baseline: 2.0668x; 2.0668x over previous
"""Trainium2 Bass kernel for BoxMultiHeadedAttention (B=8, N=512, D=512, H=8).

Sharding: data-parallel over batch — each of the 8 NeuronCores computes one
batch element end-to-end; weights replicated; no collectives.

Sparsity compaction (host-side, per call; sizes padded to the max over the
8 batch elements so a single SPMD program serves all cores):
  * keys with mask==0 contribute exp(-1e9)=0 -> dropped entirely;
    kept keys ordered [mask&obj ("geo" keys) | mask&~obj], padded to
    NRB5*128 with -1e9 mask columns.
  * queries permuted obj-first: the geometry bias only applies to
    (obj_i & obj_j) pairs, so wg is computed for geo-keys x obj-queries
    only; per-core residual regions are neutralized with data
    ([P,1] bias/clip vectors and an obj-query column mask).
  * output rows are inverse-permuted on the host.

Per-core algorithm (layout [keys(part), queries(free)] throughout):
  * x/W shipped bf16; xT via DMA-transpose (no PE transposes).
  * geometry: g = clip(ln((dx/w_i)^2), C2) on DVE+ACT; phases t = a/(4pi)*g
    via f32 selector matmul; sin/cos by exact magic-number folds
    (sin(2pi t) = Sin(-2pi*(round(t)-t)); cos via round(t+1/4) and
    bias pi/2); per-head contraction on PE (bf16); dw/dh separable
    rank-64 bank contraction as usual.
  * wg multiplier M = 1 + max(wg+bG-1, 1e-6-1)*objq masked per-core via
    [P,1] vectors; routed to attention layout through a DRAM roundtrip
    (plain-partition DMAs only).
  * exp-domain softmax: T = E * M on the geo sub-tile only; row sums via
    ones-matmul; output projection with bias folded in as a ones-row
    matmul.
"""
import math
import numpy as np
import ml_dtypes
from contextlib import ExitStack

import concourse.bass as bass
import concourse.mybir as mybir
import concourse.tile as tile
from concourse.bass_utils import run_bass_kernel_spmd

F32 = mybir.dt.float32
BF16 = mybir.dt.bfloat16
AF = mybir.ActivationFunctionType
ALU = mybir.AluOpType

B, N, D, H = 8, 512, 512, 8
DK = D // H
P = 128
NRB = N // P
GM = 16
WAVE_LEN = 1000.0
MAGIC = 12582912.0
C2 = float(2.0 * math.log(0.001))
ESHIFT = -6.0
TWO_PI = float(2.0 * math.pi)
HALF_PI = float(math.pi / 2.0)

_alphas = (100.0 / (WAVE_LEN ** (np.arange(8) / 8.0))).astype(np.float64)
BF = ml_dtypes.bfloat16


def _split_multi_waits(nc):
    """walrus accepts only ONE sync-wait per ISA instruction; hoist extras
    onto NoOps inserted before the offending instruction."""
    n_fix = 0
    for blk in nc.main_func.blocks:
        insts = list(blk.instructions)
        out, dirty = [], False
        for inst in insts:
            si = inst.sync_info
            waits = list(si.on_wait) if si is not None else []
            if len(waits) > 1:
                for kk, w in enumerate(waits[:-1]):
                    out.append(mybir.InstNoOp(
                        name=f"I-waitfix-{n_fix}-{kk}", engine=inst.engine,
                        sync_info=mybir.SyncInfo(on_wait=[w], on_update=[])))
                inst.sync_info = mybir.SyncInfo(
                    on_wait=[waits[-1]], on_update=list(si.on_update))
                n_fix += 1
                dirty = True
            out.append(inst)
        if dirty:
            blk.instructions = out
    return n_fix


def _selector_const():
    # SELAP[64*W + q*16 + m_loc, q, m_loc*8 + j] = alpha_j/(4pi)
    selap = np.zeros((P, 4, P), dtype=np.float32)
    for W in range(2):
        for q in range(4):
            for m_loc in range(GM):
                for j in range(8):
                    selap[64 * W + q * 16 + m_loc, q, m_loc * 8 + j] = \
                        _alphas[j] / (4.0 * math.pi)
    return selap


def _onehot8():
    oh = np.zeros((P, H, H), dtype=np.float32)
    for h in range(H):
        oh[:, h, h] = 1.0
    return oh


def _wblk_direct(WG):
    # direct sin/cos weights: c in (sin-x, cos-x, sin-y, cos-y)
    gmap = [lambda j: j, lambda j: 32 + j, lambda j: 8 + j, lambda j: 40 + j]
    wblk = np.zeros((4, P, P), dtype=np.float32)
    for c in range(4):
        for m_loc in range(GM):
            for j in range(8):
                for h in range(H):
                    wblk[c, m_loc * 8 + j, h * GM + m_loc] = WG[h, gmap[c](j)]
    return wblk


def _bank_consts(WG):
    # dw/dh rank-64 decomposition (sin(A-B) via quarter-phase shifts);
    # identical to the known-good formulation.
    acol = np.zeros((64, 1), np.float32)
    pcol_m = np.zeros((64, 1), np.float32)
    pcol_n = np.zeros((64, 1), np.float32)
    w1 = np.zeros((64, H), np.float32)
    for f in range(2):
        for j in range(8):
            gs = 16 + 8 * f + j
            gc = 48 + 8 * f + j
            a = _alphas[j] / (4.0 * math.pi)
            for t in range(4):
                k = (f * 8 + j) * 4 + t
                acol[k, 0] = a
                pcol_m[k, 0] = 0.25 if t in (0, 2) else 0.0
                if t == 0:
                    pcol_n[k, 0] = 0.0; w1[k] = WG[:, gs]
                elif t == 1:
                    pcol_n[k, 0] = 0.75; w1[k] = WG[:, gs]   # -cos -> +pi
                elif t == 2:
                    pcol_n[k, 0] = 0.25; w1[k] = WG[:, gc]
                else:
                    pcol_n[k, 0] = 0.0; w1[k] = WG[:, gc]
    w1e = np.repeat(w1, GM, axis=1).astype(np.float32)
    return acol, pcol_m, pcol_n, w1e


def _host_prep(inputs):
    q = np.asarray(inputs["input_query"], np.float32)
    k = np.asarray(inputs["input_key"], np.float32)
    v = np.asarray(inputs["input_value"], np.float32)
    box = np.asarray(inputs["input_box"], np.float32)
    mask = np.asarray(inputs["mask"])
    nobj = np.asarray(inputs["not_objects"])
    WG = np.asarray(inputs["WG"], np.float32)
    bG = np.asarray(inputs["bG"], np.float32)

    x_min, y_min, x_max, y_max = [box[..., i] for i in range(4)]
    cx = (x_min + x_max) * 0.5
    cy = (y_min + y_max) * 0.5
    ww = x_max - x_min + 1.0
    hh = y_max - y_min + 1.0
    l2w = (2.0 * np.log(ww)).astype(np.float32)
    l2h = (2.0 * np.log(hh)).astype(np.float32)

    keyo, qo, G5s, K5s, Q5s = [], [], [], [], []
    for b in range(B):
        m_b = mask[b] != 0
        o_b = ~nobj[b]
        geo = np.where(m_b & o_b)[0]
        oth = np.where(m_b & ~o_b)[0]
        keyo.append(np.concatenate([geo, oth]))
        qobj = np.where(o_b)[0]
        qrest = np.where(~o_b)[0]
        qo.append(np.concatenate([qobj, qrest]))
        G5s.append(len(geo)); K5s.append(len(geo) + len(oth))
        Q5s.append(len(qobj))

    G5max = max(max(G5s), 1)
    n_geo = (G5max + GM - 1) // GM
    G5pad = n_geo * GM
    GBLK = (G5pad + P - 1) // P
    K5max = max(max(K5s), 1)
    NRB5 = (K5max + P - 1) // P
    K5pad = NRB5 * P
    Q5max = max(max(Q5s), 1)
    Q5pad = min(N, ((Q5max + 31) // 32) * 32)
    sizes = (n_geo, GBLK, NRB5, Q5pad)

    acol, pcol_m, pcol_n, w1e = _bank_consts(WG)
    shared = {
        "Wq": np.asarray(inputs["Wq"], np.float32).astype(BF),
        "Wk": np.asarray(inputs["Wk"], np.float32).astype(BF),
        "Wv": np.asarray(inputs["Wv"], np.float32).astype(BF),
        "Wo": np.asarray(inputs["Wo"], np.float32).astype(BF),
        "bqcol": np.asarray(inputs["bq"], np.float32).reshape(NRB, P).T.copy(),
        "bk8col": (np.asarray(inputs["bk"], np.float32) * 8.0
                   ).reshape(NRB, P).T.copy(),
        "bvrow": np.asarray(inputs["bv"], np.float32).astype(BF)[None, :],
        "borow": np.asarray(inputs["bo"], np.float32).astype(BF)[None, :],
        "SELAP": _selector_const(),
        "ONEHOT8": _onehot8().astype(BF),
        "WBLK": _wblk_direct(WG).astype(BF),
        "W1E": w1e, "ACOL": acol, "PCOL_M": pcol_m, "PCOL_N": pcol_n,
    }

    in_maps = []
    for b in range(B):
        ko, qp = keyo[b], qo[b]
        G5, K5, Q5 = G5s[b], K5s[b], Q5s[b]

        xq_p = q[b][qp].astype(BF)
        xk_c = np.zeros((K5pad, D), BF)
        xk_c[:K5] = k[b][ko].astype(BF)
        xv_c = np.zeros((K5pad, D), BF)
        xv_c[:K5] = v[b][ko].astype(BF)

        # key-side geometry (geo order, padded benign)
        cxk = np.zeros(GBLK * P, np.float32); cxk[:G5] = cx[b][ko[:G5]]
        cyk = np.zeros(GBLK * P, np.float32); cyk[:G5] = cy[b][ko[:G5]]
        l2wk = np.zeros(G5pad, np.float32); l2wk[:G5] = l2w[b][ko[:G5]]
        l2hk = np.zeros(G5pad, np.float32); l2hk[:G5] = l2h[b][ko[:G5]]
        # query-side (perm order, truncated to Q5pad, padded benign)
        cxq = np.zeros(Q5pad, np.float32)
        cyq = np.zeros(Q5pad, np.float32)
        iwq = np.ones(Q5pad, np.float32)
        ihq = np.ones(Q5pad, np.float32)
        l2wq = np.zeros(Q5pad, np.float32)
        l2hq = np.zeros(Q5pad, np.float32)
        nq = min(Q5pad, N)
        cxq[:nq] = cx[b][qp[:nq]]; cyq[:nq] = cy[b][qp[:nq]]
        iwq[:nq] = 1.0 / ww[b][qp[:nq]]; ihq[:nq] = 1.0 / hh[b][qp[:nq]]
        l2wq[:nq] = l2w[b][qp[:nq]]; l2hq[:nq] = l2h[b][qp[:nq]]
        objq = np.zeros(Q5pad, np.float32)
        objq[:min(Q5, Q5pad)] = 1.0

        maskcol = np.full((P, NRB5), -1e9 + ESHIFT, np.float32)
        mc = maskcol.T.reshape(-1)
        mc[:K5] = ESHIFT
        maskcol = mc.reshape(NRB5, P).T.copy()

        bgm1 = np.zeros((P, n_geo), np.float32)
        epsm1 = np.zeros((P, n_geo), np.float32)
        for g in range(n_geo):
            for m in range(GM):
                key = g * GM + m
                for h in range(H):
                    if key < G5:
                        bgm1[h * GM + m, g] = bG[h] - 1.0
                        epsm1[h * GM + m, g] = 1e-6 - 1.0
                    else:
                        bgm1[h * GM + m, g] = -1e9
                        epsm1[h * GM + m, g] = 0.0

        mm = dict(shared)
        mm.update({
            "xq": xq_p, "xk": xk_c, "xv": xv_c,
            "cxk": cxk.reshape(GBLK, P).T.copy(),
            "cyk": cyk.reshape(GBLK, P).T.copy(),
            "l2wk": l2wk, "l2hk": l2hk,
            "cxq": cxq, "cyq": cyq, "iwq": iwq, "ihq": ihq,
            "l2wq": l2wq, "l2hq": l2hq,
            "objq": objq.astype(BF),
            "mcol": maskcol, "bgm1": bgm1, "epsm1": epsm1,
        })
        in_maps.append(mm)

    inv_q = [np.argsort(qp) for qp in qo]
    return in_maps, sizes, inv_q


def build_nc(n_geo, GBLK, NRB5, Q5pad):
    K5pad = NRB5 * P
    G5pad = n_geo * GM
    nc = bass.Bass()

    def dp(name, shape, dt=F32):
        return nc.declare_dram_parameter(name, list(shape), dt, isOutput=False)

    xq = dp("xq", (N, D), BF16)
    xk = dp("xk", (K5pad, D), BF16)
    xv = dp("xv", (K5pad, D), BF16)
    Wq = dp("Wq", (D, D), BF16); Wk = dp("Wk", (D, D), BF16)
    Wv = dp("Wv", (D, D), BF16); Wo = dp("Wo", (D, D), BF16)
    bqcol = dp("bqcol", (P, NRB)); bk8col = dp("bk8col", (P, NRB))
    bvrow = dp("bvrow", (1, D), BF16); borow = dp("borow", (1, D), BF16)
    SELAP = dp("SELAP", (P, 4, P))
    ONEHOT8 = dp("ONEHOT8", (P, H, H), BF16)
    WBLK = dp("WBLK", (4, P, P), BF16)
    W1E = dp("W1E", (64, P))
    ACOL = dp("ACOL", (64, 1))
    PCOL_M = dp("PCOL_M", (64, 1)); PCOL_N = dp("PCOL_N", (64, 1))
    cxk = dp("cxk", (P, GBLK)); cyk = dp("cyk", (P, GBLK))
    l2wk = dp("l2wk", (G5pad,)); l2hk = dp("l2hk", (G5pad,))
    cxq = dp("cxq", (Q5pad,)); cyq = dp("cyq", (Q5pad,))
    iwq = dp("iwq", (Q5pad,)); ihq = dp("ihq", (Q5pad,))
    l2wq = dp("l2wq", (Q5pad,)); l2hq = dp("l2hq", (Q5pad,))
    objq = dp("objq", (Q5pad,), BF16)
    mcol = dp("mcol", (P, NRB5))
    bgm1 = dp("bgm1", (P, n_geo)); epsm1 = dp("epsm1", (P, n_geo))
    out = nc.declare_dram_parameter("out", [N, D], F32, isOutput=True)
    wgd_dram = nc.dram_tensor("wgd_scratch", [n_geo, H, GM, Q5pad], BF16)
    rs_dram = nc.dram_tensor("rs_scratch", [H, N], F32)

    with ExitStack() as ctx:
        tc = ctx.enter_context(tile.TileContext(nc))
        const = ctx.enter_context(tc.tile_pool(name="const", bufs=1))
        persist = ctx.enter_context(tc.tile_pool(name="persist", bufs=1))

        # ---------------- constants ----------------
        selap_t = const.tile([P, 4, P], F32, tag="selap")
        nc.sync.dma_start(selap_t[:], SELAP[:])
        oh8_t = const.tile([P, H, H], BF16, tag="oh8")
        nc.sync.dma_start(oh8_t[:], ONEHOT8[:])
        wblk_t4 = []
        for c in range(4):
            wb = const.tile([P, P], BF16, tag=f"wblkb{c}")
            nc.sync.dma_start(wb[:], WBLK[c])
            wblk_t4.append(wb)
        w1e_f = const.tile([64, P], F32, tag="w1e")
        nc.sync.dma_start(w1e_f[:], W1E[:])
        acol_t = const.tile([64, 1], F32, tag="acol")
        nc.sync.dma_start(acol_t[:], ACOL[:])
        pcolm_t = const.tile([64, 1], F32, tag="pcolm")
        nc.sync.dma_start(pcolm_t[:], PCOL_M[:])
        pcoln_t = const.tile([64, 1], F32, tag="pcoln")
        nc.sync.dma_start(pcoln_t[:], PCOL_N[:])
        halfpi_t = const.tile([P, 1], F32, tag="halfpi")
        nc.vector.memset(halfpi_t[:], HALF_PI)
        ones1_bf = const.tile([1, P], BF16, tag="ones1")
        nc.vector.memset(ones1_bf[:], 1.0)
        bvrow_t = const.tile([1, D], BF16, tag="bvrow")
        nc.sync.dma_start(bvrow_t[:], bvrow[:])
        borow_t = const.tile([1, D], BF16, tag="borow")
        nc.sync.dma_start(borow_t[:], borow[:])
        mcol_t = const.tile([P, NRB5], F32, tag="mcol")
        nc.sync.dma_start(mcol_t[:], mcol[:])
        bq_t = const.tile([P, NRB], F32, tag="bq")
        nc.sync.dma_start(bq_t[:], bqcol[:])
        bk8_t = const.tile([P, NRB], F32, tag="bk8")
        nc.sync.dma_start(bk8_t[:], bk8col[:])
        cxk_t = const.tile([P, GBLK], F32, tag="cxk")
        nc.sync.dma_start(cxk_t[:], cxk[:])
        cyk_t = const.tile([P, GBLK], F32, tag="cyk")
        nc.sync.dma_start(cyk_t[:], cyk[:])
        bgm1_t = const.tile([P, n_geo], F32, tag="bgm1")
        nc.sync.dma_start(bgm1_t[:], bgm1[:])
        epsm1_t = const.tile([P, n_geo], F32, tag="epsm1")
        nc.sync.dma_start(epsm1_t[:], epsm1[:])
        cxqbc = const.tile([P, Q5pad], F32, tag="cxqbc")
        nc.sync.dma_start(cxqbc[:], cxq[None, :].to_broadcast((P, Q5pad)))
        cyqbc = const.tile([P, Q5pad], F32, tag="cyqbc")
        nc.sync.dma_start(cyqbc[:], cyq[None, :].to_broadcast((P, Q5pad)))
        iwqbc = const.tile([P, Q5pad], F32, tag="iwqbc")
        nc.sync.dma_start(iwqbc[:], iwq[None, :].to_broadcast((P, Q5pad)))
        ihqbc = const.tile([P, Q5pad], F32, tag="ihqbc")
        nc.sync.dma_start(ihqbc[:], ihq[None, :].to_broadcast((P, Q5pad)))
        objqbc = const.tile([P, Q5pad], BF16, tag="objqbc")
        nc.sync.dma_start(objqbc[:], objq[None, :].to_broadcast((P, Q5pad)))
        # bank l2 tiles: [0:32]=w, [32:64]=h
        l2kM = const.tile([64, G5pad], F32, tag="l2kM")
        nc.sync.dma_start(l2kM[:32, :], l2wk[None, :].to_broadcast((32, G5pad)))
        nc.sync.dma_start(l2kM[32:, :], l2hk[None, :].to_broadcast((32, G5pad)))
        l2qN = const.tile([64, Q5pad], F32, tag="l2qN")
        nc.sync.dma_start(l2qN[:32, :], l2wq[None, :].to_broadcast((32, Q5pad)))
        nc.sync.dma_start(l2qN[32:, :], l2hq[None, :].to_broadcast((32, Q5pad)))

        # ---------------- phase 2: ln fields (geo keys x obj queries) -------
        dxy2 = persist.tile([P, GBLK, 2, Q5pad], F32, tag="dxy2")
        with tc.tile_pool(name="work2", bufs=3) as work2:
            for blk in range(GBLK):
                for (ci, cbc, ccol, ibc) in ((0, cxqbc, cxk_t, iwqbc),
                                             (1, cyqbc, cyk_t, ihqbc)):
                    d_ = work2.tile([P, Q5pad], F32, tag="geo_d")
                    nc.vector.tensor_scalar(d_[:], cbc[:],
                                            ccol[:, blk:blk + 1], None,
                                            ALU.subtract)
                    dw_ = work2.tile([P, Q5pad], F32, tag="geo_dw")
                    nc.vector.tensor_tensor(dw_[:], d_[:], ibc[:], ALU.mult)
                    d2 = work2.tile([P, Q5pad], F32, tag="geo_d2")
                    nc.scalar.activation(d2[:], dw_[:], AF.Square)
                    l2t = work2.tile([P, Q5pad], F32, tag="geo_l2")
                    nc.scalar.activation(l2t[:], d2[:], AF.Ln)
                    nc.vector.tensor_scalar_max(dxy2[:, blk, ci, :], l2t[:],
                                                C2)

        # ---------------- phase 3: dw/dh banks ----------------
        bankM = persist.tile([64, G5pad], BF16, tag="bankM")
        bankN = persist.tile([64, Q5pad], BF16, tag="bankN")
        with tc.tile_pool(name="work3", bufs=2) as work3:
            for (pcol, l2bc, width, bank) in ((pcolm_t, l2kM, G5pad, bankM),
                                              (pcoln_t, l2qN, Q5pad, bankN)):
                t_ = work3.tile([64, width], F32, tag="bk_t")
                nc.vector.tensor_scalar(t_[:], l2bc[:], acol_t[:], pcol[:],
                                        ALU.mult, ALU.add)
                r_ = work3.tile([64, width], F32, tag="bk_r")
                nc.vector.tensor_scalar(r_[:], t_[:], MAGIC, -MAGIC,
                                        ALU.add, ALU.add)
                f_ = work3.tile([64, width], F32, tag="bk_f")
                nc.vector.tensor_tensor(f_[:], t_[:], r_[:], ALU.subtract)
                nc.scalar.activation(bank[:], f_[:], AF.Sin, scale=TWO_PI)

        # ---------------- phase 1: DMA-transpose loads + projections --------
        xqT = persist.tile([P, NRB, N], BF16, tag="xqT")
        xkT = persist.tile([P, NRB, K5pad], BF16, tag="xkT")
        xvT = persist.tile([P, NRB, K5pad], BF16, tag="xvT")
        wq_b = persist.tile([P, NRB, D], BF16, tag="wqb")
        wk_b = persist.tile([P, NRB, D], BF16, tag="wkb")
        wv_b = persist.tile([P, NRB, D], BF16, tag="wvb")
        wo_b = persist.tile([P, NRB, D], BF16, tag="wob")
        qT = persist.tile([P, NRB, N], BF16, tag="qT")
        kTt = persist.tile([P, NRB, K5pad], BF16, tag="kT")
        v_sb = persist.tile([P, NRB5, D], BF16, tag="v_sb")

        for cb in range(NRB):
            nc.sync.dma_start_transpose(xqT[:, cb, :],
                                        xq[:, cb * P:(cb + 1) * P])
            nc.sync.dma_start_transpose(xkT[:, cb, :],
                                        xk[:, cb * P:(cb + 1) * P])
            nc.sync.dma_start_transpose(xvT[:, cb, :],
                                        xv[:, cb * P:(cb + 1) * P])
        for (Wd, wb_) in ((Wq, wq_b), (Wk, wk_b), (Wv, wv_b), (Wo, wo_b)):
            nc.sync.dma_start(wb_[:], Wd.rearrange("(kb p) d -> p kb d", p=P))

        with tc.tile_pool(name="psum1", bufs=2, space="PSUM") as psum1:
            for ob in range(NRB):
                ps = psum1.tile([P, N], F32, tag="qps")
                for kb in range(NRB):
                    nc.tensor.matmul(ps[:], wq_b[:, kb, ob * P:(ob + 1) * P],
                                     xqT[:, kb, :],
                                     start=(kb == 0), stop=(kb == NRB - 1))
                nc.vector.tensor_scalar(qT[:, ob, :], ps[:],
                                        bq_t[:, ob:ob + 1], None, ALU.add)
            for ob in range(NRB):
                ps = psum1.tile([P, K5pad], F32, tag="kps")
                for kb in range(NRB):
                    nc.tensor.matmul(ps[:], wk_b[:, kb, ob * P:(ob + 1) * P],
                                     xkT[:, kb, :],
                                     start=(kb == 0), stop=(kb == NRB - 1))
                nc.vector.tensor_scalar(kTt[:, ob, :], ps[:],
                                        bk8_t[:, ob:ob + 1], 0.125,
                                        ALU.add, ALU.mult)
            for mb in range(NRB5):
                ps = psum1.tile([P, D], F32, tag="vps")
                for kb in range(NRB):
                    nc.tensor.matmul(ps[:], xvT[:, kb, mb * P:(mb + 1) * P],
                                     wv_b[:, kb, :],
                                     start=(kb == 0), stop=False)
                nc.tensor.matmul(ps[:], ones1_bf[:], bvrow_t[:],
                                 start=False, stop=True)
                nc.scalar.copy(v_sb[:, mb, :], ps[:])

        # ---------------- phase 4: geometry weights ----------------
        wgdT = persist.tile([P, GBLK, H, Q5pad], BF16, tag="wgdT")
        with tc.tile_pool(name="work4", bufs=3) as work4, \
             tc.tile_pool(name="psum_u", bufs=2, space="PSUM") as psum_u, \
             tc.tile_pool(name="psum_wg", bufs=2, space="PSUM") as psum_wg:
            for g in range(n_geo):
                blk = g // 8
                off = 64 * ((g % 8) // 4)
                q4 = g % 4
                mbase = g * GM
                lhs_wh = work4.tile([64, P], BF16, tag="lhs_wh")
                nc.vector.tensor_tensor(
                    lhs_wh[:].rearrange("k (h m) -> k h m", h=H),
                    w1e_f[:].rearrange("k (h m) -> k h m", h=H),
                    bankM[:, mbase:mbase + GM][:, None, :]
                        .to_broadcast((64, H, GM)),
                    ALU.mult)
                ups = psum_u.tile([P, 2, N], F32, tag="ups")
                for ci in range(2):
                    nc.tensor.matmul(ups[:, ci, :Q5pad],
                                     selap_t[off:off + 64, q4, :],
                                     dxy2[off:off + 64, blk, ci, :],
                                     start=True, stop=True)
                upsv = ups[:, :, :Q5pad]
                rrS = work4.tile([P, 2, Q5pad], F32, tag="rrS")
                nc.vector.tensor_scalar(rrS[:], upsv, MAGIC, -MAGIC,
                                        ALU.add, ALU.add)
                nfS = work4.tile([P, 2, Q5pad], F32, tag="nfS")
                nc.vector.tensor_tensor(nfS[:], rrS[:], upsv, ALU.subtract)
                rrC = work4.tile([P, 2, Q5pad], F32, tag="rrC")
                nc.vector.tensor_scalar(rrC[:], upsv, 0.25, MAGIC,
                                        ALU.add, ALU.add)
                rrC2 = work4.tile([P, 2, Q5pad], F32, tag="rrC2")
                nc.gpsimd.tensor_scalar(rrC2[:], rrC[:], -MAGIC, None, ALU.add)
                nfC = work4.tile([P, 2, Q5pad], F32, tag="nfC")
                nc.vector.tensor_tensor(nfC[:], rrC2[:], upsv, ALU.subtract)
                sS = work4.tile([P, 2, Q5pad], BF16, tag="sS")
                nc.scalar.activation(sS[:], nfS[:], AF.Sin, scale=-TWO_PI)
                sC = work4.tile([P, 2, Q5pad], BF16, tag="sC")
                nc.scalar.activation(sC[:], nfC[:], AF.Sin, scale=-TWO_PI,
                                     bias=halfpi_t[:])
                wgp = psum_wg.tile([P, N], F32, tag="wgp")
                nc.tensor.matmul(wgp[:, :Q5pad], wblk_t4[0][:], sS[:, 0, :],
                                 start=True, stop=False)
                nc.tensor.matmul(wgp[:, :Q5pad], wblk_t4[1][:], sC[:, 0, :],
                                 start=False, stop=False)
                nc.tensor.matmul(wgp[:, :Q5pad], wblk_t4[2][:], sS[:, 1, :],
                                 start=False, stop=False)
                nc.tensor.matmul(wgp[:, :Q5pad], wblk_t4[3][:], sC[:, 1, :],
                                 start=False, stop=False)
                nc.tensor.matmul(wgp[:, :Q5pad], lhs_wh[:], bankN[:],
                                 start=False, stop=True)
                wgdB = work4.tile([P, Q5pad], BF16, tag="wgdB")
                nc.vector.tensor_scalar(wgdB[:], wgp[:, :Q5pad],
                                        bgm1_t[:, g:g + 1],
                                        epsm1_t[:, g:g + 1],
                                        ALU.add, ALU.max)
                wgdm1 = work4.tile([P, Q5pad], BF16, tag="wgdm1")
                nc.gpsimd.tensor_tensor(wgdm1[:], wgdB[:], objqbc[:],
                                        ALU.mult)
                wgdM = work4.tile([P, Q5pad], BF16, tag="wgdM")
                nc.vector.tensor_scalar(wgdM[:], wgdm1[:], 1.0, None, ALU.add)
                nc.scalar.dma_start(
                    wgd_dram[g].rearrange("h t q -> (h t) q"), wgdM[:])
            # gather to attention layout: [key(part), blk, h, q]
            for blk in range(GBLK):
                gcnt = min(8, n_geo - blk * 8)
                for h in range(H):
                    nc.scalar.dma_start(
                        wgdT[0:gcnt * GM, blk, h, :],
                        wgd_dram[blk * 8:blk * 8 + gcnt, h, :, :])

        # ---------------- phase 5: attention ----------------
        ot = persist.tile([P, NRB, N], BF16, tag="ot")
        with tc.tile_pool(name="work5", bufs=3) as work5, \
             tc.tile_pool(name="psum5", bufs=2, space="PSUM") as psum5, \
             tc.tile_pool(name="psum_s", bufs=2, space="PSUM") as psum_s, \
             tc.tile_pool(name="psum_av", bufs=2, space="PSUM") as psum_av:
            for ob in range(NRB):
                h0 = 2 * ob
                av = psum_av.tile([P, N], F32, tag="avps")
                sbank = psum_s.tile([H, N], F32, tag="sbank")
                for rb in range(NRB5):
                    st2 = psum5.tile([P, 2, N], F32, tag="stps")
                    for hi in range(2):
                        po = hi * DK
                        nc.tensor.matmul(
                            st2[:, hi, :],
                            kTt[po:po + DK, ob, rb * P:(rb + 1) * P],
                            qT[po:po + DK, ob, :], start=True, stop=True)
                    e_ = work5.tile([P, 2, N], BF16, tag="e_t")
                    nc.scalar.activation(e_[:], st2[:], AF.Exp,
                                         bias=mcol_t[:, rb:rb + 1])
                    if rb < GBLK:
                        rows = min(P, G5pad - rb * P)
                        nc.vector.tensor_tensor(
                            e_[0:rows, :, 0:Q5pad], e_[0:rows, :, 0:Q5pad],
                            wgdT[0:rows, rb, h0:h0 + 2, :], ALU.mult)
                    for hi in range(2):
                        po = hi * DK
                        nc.tensor.matmul(sbank[:], oh8_t[:, h0 + hi, :],
                                         e_[:, hi, :],
                                         start=(rb == 0 and hi == 0),
                                         stop=(rb == NRB5 - 1 and hi == 1),
                                         skip_group_check=True)
                        nc.tensor.matmul(av[po:po + DK, :],
                                         v_sb[:, rb,
                                              (h0 + hi) * DK:(h0 + hi + 1) * DK],
                                         e_[:, hi, :], start=(rb == 0),
                                         stop=(rb == NRB5 - 1),
                                         skip_group_check=True)
                rs = work5.tile([H, N], F32, tag="rs")
                nc.vector.reciprocal(rs[:], sbank[:])
                nc.sync.dma_start(rs_dram[h0:h0 + 2, :], rs[h0:h0 + 2, :])
                rr_b = work5.tile([P, N], F32, tag="rr_b")
                for hi in range(2):
                    nc.sync.dma_start(
                        rr_b[hi * DK:(hi + 1) * DK, :],
                        rs_dram[h0 + hi:h0 + hi + 1, :].to_broadcast((DK, N)))
                nc.vector.tensor_tensor(ot[:, ob, :], av[:], rr_b[:], ALU.mult)

        # ---------------- phase 6: output projection ----------------
        with tc.tile_pool(name="work6", bufs=2) as work6, \
             tc.tile_pool(name="psum6", bufs=2, space="PSUM") as psum6:
            for r in range(NRB):
                ps = psum6.tile([P, D], F32, tag="fps")
                for kt in range(NRB):
                    nc.tensor.matmul(ps[:], ot[:, kt, r * P:(r + 1) * P],
                                     wo_b[:, kt, :],
                                     start=(kt == 0), stop=False)
                nc.tensor.matmul(ps[:], ones1_bf[:], borow_t[:],
                                 start=False, stop=True)
                fo = work6.tile([P, D], F32, tag="fo")
                nc.scalar.copy(fo[:], ps[:])
                nc.sync.dma_start(out[r * P:(r + 1) * P, :], fo[:])

    _split_multi_waits(nc)
    return nc


_NC_CACHE = {}


def kernel(**inputs):
    in_maps, sizes, inv_q = _host_prep(inputs)
    if _NC_CACHE.get("sizes") != sizes:
        _NC_CACHE["nc"] = build_nc(*sizes)
        _NC_CACHE["sizes"] = sizes
    nc = _NC_CACHE["nc"]
    res = run_bass_kernel_spmd(nc, in_maps, list(range(B)))
    out = np.stack([res.results[b]["out"][inv_q[b]] for b in range(B)], axis=0)
    return out.astype(np.float32)


if __name__ == "__main__":
    print("kernel module ok")
